# revision 10
# baseline (speedup 1.0000x reference)
"""CovQuadraticCrossEntropyLoss Trainium2 kernel.

Reference computation, per (s, b) pair with V = 512:
    p    = softmax(m)                                  [V]
    quad = 0.5 * (sum_i K_ii p_i - p^T K p)
    ce   = logsumexp(m) - m[target]
    loss = ce + quad

Strategy (memory-bound: k is 512 MB total, 64 MB per core; the roofline is
the ~179 us it takes one core to stream 64 MB of K from HBM):
  - Fully data-parallel over the s axis: core i handles s in [4i, 4i+4),
    i.e. 64 (s, b) slabs of K [512, 512] each.
  - One fused softmax pass over m [64, 512] gives e = exp(m - max) and
    Z = sum(e) (p = e / Z is never materialized; the p factors are divided
    out at the end: p^T K p = e^T K e / Z^2).
  - K slab row i lands on partition i//4, chunk i%4, so each partition's
    slab data is one contiguous 8 KB run of HBM: 128 descriptors per 1 MB
    slab DMA instead of 512.
  - Per slab: 4 accumulating float32r matmuls x[1, 512] += eT_c^T @ K_c
    compute x = K^T e at 1 cycle/row while K streams, then one DVE
    tensor_tensor_reduce against a single-partition copy of e reads x
    straight out of PSUM and lands the dot e^T K e in t_cols[0, s].
  - diag(K) is extracted host-side (a [SLABS, V] strided copy, 0.2% of K's
    bytes) and shipped as its own input: a device-side 4-byte-granule gather
    costs 32K DMA descriptors that contend with the K stream.
  - Everything not depending on x (ce, the diag term, 0.5/Z^2) is computed
    early in [64, 1] form and PE-transposed to rows, so the tail after the
    last matmul is one dot + two [1, 64] DVE ops + a 256 B output DMA.
"""

import numpy as np

import concourse.bass as bass
import concourse.mybir as mybir
import concourse.tile as tile
from concourse.masks import make_identity

S, B, V = 32, 16, 512
N_CORES = 8
S_PER_CORE = S // N_CORES          # 4
SLABS = S_PER_CORE * B             # 64 (s, b) pairs per core
P = 128                            # partitions
CHUNKS = V // P                    # 4
F32 = mybir.dt.float32
F32R = mybir.dt.float32r


def _split_multi_wait_instructions(nc: bass.Bass) -> None:
    """Rewrite the BIR so no instruction carries more than one sem wait.

    The walrus build here rejects instructions with >1 sync-wait command
    ("Too many sync wait commands", CoreV3GenImpl setupSyncWait). Engines
    execute their streams in order, so an instruction's extra waits can be
    moved onto same-engine NOPs inserted immediately before it.
    """
    for fn in nc.m.functions:
        for bb in fn.blocks:
            new_insts = []
            for inst in bb.instructions:
                si = inst.sync_info
                waits = list(si.on_wait) if si is not None and si.on_wait else []
                if len(waits) > 1:
                    for j, w in enumerate(waits[:-1]):
                        new_insts.append(
                            mybir.InstNoOp(
                                name=f"{inst.name}-sw{j}",
                                engine=inst.engine,
                                bass_nofuse=True,
                                sync_info=mybir.SyncInfo(on_wait=[w], on_update=[]),
                            )
                        )
                    inst.sync_info = mybir.SyncInfo(
                        on_wait=[waits[-1]],
                        on_update=list(si.on_update or []),
                    )
                new_insts.append(inst)
            bb.instructions = new_insts


def build_bass(k_bufs: int = 7, x_bufs: int = 6) -> bass.Bass:
    nc = bass.Bass(name="covq_ce")
    m_d = nc.dram_tensor("m", [SLABS, V], F32, kind="ExternalInput")
    k_d = nc.dram_tensor("k", [SLABS, V, V], F32, kind="ExternalInput")
    diag_d = nc.dram_tensor("diag", [SLABS, V], F32, kind="ExternalInput")
    tgt_d = nc.dram_tensor("tgt", [SLABS, 1], F32, kind="ExternalInput")
    out_d = nc.dram_tensor("out", [1, SLABS], F32, kind="ExternalOutput")

    # K slab s as [partition p, chunk c, j] with row index i = p*4 + c, so
    # partition p's line (rows 4p..4p+3) is 8 KB of contiguous HBM.
    k_r = k_d[:, :, :].rearrange("n (p c) j -> n p c j", p=P)

    with tile.TileContext(nc) as tc:
        with (
            tc.tile_pool(name="singles", bufs=1) as singles,
            tc.tile_pool(name="kpool", bufs=k_bufs) as kpool,
            tc.tile_pool(name="psum_t", bufs=1, space="PSUM") as psum_t,
            tc.tile_pool(name="psum_x", bufs=x_bufs, space="PSUM") as psum_x,
        ):
            # --- small inputs: m on the fast HWDGE ring (ahead of the K
            # stream), the rest on SWDGE; iota is generated on-chip.
            m_sb = singles.tile([SLABS, V], F32)
            nc.sync.dma_start(out=m_sb, in_=m_d[:, :])

            identity = singles.tile([P, P], F32)
            make_identity(nc, identity)

            diag_sb = singles.tile([SLABS, V], F32)
            nc.gpsimd.dma_start(out=diag_sb, in_=diag_d[:, :])
            iota_sb = singles.tile([SLABS, V], F32)
            nc.gpsimd.iota(
                iota_sb,
                pattern=[[1, V]],
                base=0,
                channel_multiplier=0,
                allow_small_or_imprecise_dtypes=True,
            )
            tgt_sb = singles.tile([SLABS, 1], F32)
            nc.gpsimd.dma_start(out=tgt_sb, in_=tgt_d[:, :])

            # --- softmax pieces: e = exp(m - max), Z = sum(e) --------------
            mx = singles.tile([SLABS, 1], F32)
            nc.vector.tensor_reduce(
                out=mx, in_=m_sb, axis=mybir.AxisListType.X, op=mybir.AluOpType.max
            )
            neg_mx = singles.tile([SLABS, 1], F32)
            nc.vector.tensor_scalar_mul(out=neg_mx, in0=mx, scalar1=-1.0)
            e_sb = singles.tile([SLABS, V], F32)
            z_sb = singles.tile([SLABS, 1], F32)
            nc.scalar.activation(
                out=e_sb,
                in_=m_sb,
                func=mybir.ActivationFunctionType.Exp,
                bias=neg_mx,
                scale=1.0,
                accum_out=z_sb,
            )
            ln_z = singles.tile([SLABS, 1], F32)
            nc.scalar.activation(out=ln_z, in_=z_sb, func=mybir.ActivationFunctionType.Ln)
            inv_z = singles.tile([SLABS, 1], F32)
            nc.vector.reciprocal(out=inv_z, in_=z_sb)

            # --- transpose e -> eT[p, c, s], eT_c[p, s] = e[s, 4p+c] -------
            # (matches the i = 4p+c row layout of the K tiles). float32r so
            # the matmuls take the 1-cycle/row fp32 path.
            eT_sb = singles.tile([P, CHUNKS, SLABS], F32R)
            eT_ps = psum_t.tile([P, CHUNKS, SLABS], F32)
            for c in range(CHUNKS):
                nc.tensor.transpose(
                    eT_ps[:, c, :],
                    e_sb[:, c :: CHUNKS],
                    identity[:SLABS, :SLABS],
                )
            nc.vector.tensor_copy(eT_sb, eT_ps)

            # Single-partition copy of e: row s at [0, s*V:(s+1)*V]. Lets the
            # per-slab dot product read x straight from PSUM partition 0.
            e_flat = singles.tile([1, SLABS * V], F32)
            nc.gpsimd.dma_start(out=e_flat, in_=e_sb)

            # --- early epilogue: everything that doesn't need x ------------
            # loss = (mx + lnZ - m[tgt]) + 0.5*invZ*dq - (0.5*invZ^2) * e'Ke
            #      = a - b * t      with t = e^T K e accumulated per slab.
            scratch = singles.tile([SLABS, V], F32)
            msk = singles.tile([SLABS, V], F32)
            nc.vector.tensor_scalar(
                out=msk,
                in0=iota_sb,
                scalar1=tgt_sb,
                scalar2=None,
                op0=mybir.AluOpType.is_equal,
            )
            g = singles.tile([SLABS, 1], F32)
            nc.vector.tensor_mul(out=scratch, in0=msk, in1=m_sb)
            nc.vector.tensor_reduce(out=g, in_=scratch, axis=mybir.AxisListType.X, op=mybir.AluOpType.add)
            dq = singles.tile([SLABS, 1], F32)
            nc.vector.tensor_mul(out=scratch, in0=diag_sb, in1=e_sb)
            nc.vector.tensor_reduce(out=dq, in_=scratch, axis=mybir.AxisListType.X, op=mybir.AluOpType.add)

            ce1 = singles.tile([SLABS, 1], F32)
            nc.vector.tensor_add(out=ce1, in0=mx, in1=ln_z)
            ce2 = singles.tile([SLABS, 1], F32)
            nc.vector.tensor_sub(out=ce2, in0=ce1, in1=g)
            hdq = singles.tile([SLABS, 1], F32)
            nc.vector.tensor_mul(out=hdq, in0=dq, in1=inv_z)
            hdq2 = singles.tile([SLABS, 1], F32)
            nc.vector.tensor_scalar_mul(out=hdq2, in0=hdq, scalar1=0.5)
            a_col = singles.tile([SLABS, 1], F32)
            nc.vector.tensor_add(out=a_col, in0=ce2, in1=hdq2)
            iz2 = singles.tile([SLABS, 1], F32)
            nc.vector.tensor_mul(out=iz2, in0=inv_z, in1=inv_z)
            b_col = singles.tile([SLABS, 1], F32)
            nc.vector.tensor_scalar_mul(out=b_col, in0=iz2, scalar1=0.5)

            # Rows on partition 0 (PE transpose) for the [1, 64] tail math.
            ab_row_ps = psum_t.tile([1, 2 * SLABS], F32, tag="abrow")
            a_row_ps = ab_row_ps[:, :SLABS]
            b_row_ps = ab_row_ps[:, SLABS:]
            nc.tensor.transpose(a_row_ps, a_col, identity[:SLABS, :SLABS])
            nc.tensor.transpose(b_row_ps, b_col, identity[:SLABS, :SLABS])

            # --- main loop: stream K, t[s] = e^T K e -----------------------
            t_cols = singles.tile([1, SLABS], F32)
            ttr_out = singles.tile([1, V], F32)
            for s in range(SLABS):
                kt = kpool.tile([P, CHUNKS, V], F32R, tag="kt")
                nc.sync.dma_start(out=kt, in_=k_r[s].bitcast(F32R))
                x_ps = psum_x.tile([1, V], F32, tag="x")
                for c in range(CHUNKS):
                    nc.tensor.matmul(
                        x_ps,
                        eT_sb[:, c, s : s + 1],
                        kt[:, c, :],
                        start=(c == 0),
                        stop=(c == CHUNKS - 1),
                    )
                nc.vector.tensor_mul(
                    out=ttr_out, in0=x_ps, in1=e_flat[:, s * V : (s + 1) * V]
                )
                nc.vector.tensor_reduce(
                    out=t_cols[:, s : s + 1],
                    in_=ttr_out,
                    axis=mybir.AxisListType.X,
                    op=mybir.AluOpType.add,
                )

            # --- tail: loss_row = a_row - b_row * t ------------------------
            bt = singles.tile([1, SLABS], F32)
            nc.vector.tensor_mul(out=bt, in0=b_row_ps, in1=t_cols)
            loss_row = singles.tile([1, SLABS], F32)
            nc.vector.tensor_sub(out=loss_row, in0=a_row_ps, in1=bt)

            nc.sync.dma_start(out=out_d[:, :], in_=loss_row)

    _split_multi_wait_instructions(nc)
    return nc


_NC_CACHE = {}


def _get_nc():
    if "nc" not in _NC_CACHE:
        _NC_CACHE["nc"] = build_bass()
    return _NC_CACHE["nc"]


def run_sharded(m, k, target, trace=False, **run_kwargs):
    """Shard full inputs over 8 cores, run the bass kernel, gather output.

    Returns (loss [S, B] f32, BassKernelResults).
    """
    from concourse.bass_utils import run_bass_kernel_spmd

    m = np.ascontiguousarray(np.asarray(m), dtype=np.float32)
    k = np.ascontiguousarray(np.asarray(k), dtype=np.float32)
    target = np.asarray(target)
    assert m.shape == (S, B, V) and k.shape == (S, B, V, V)
    tgt_f = target.astype(np.float32).reshape(S, B)
    diag = np.ascontiguousarray(
        np.diagonal(k, axis1=-2, axis2=-1), dtype=np.float32
    )

    in_maps = []
    for c in range(N_CORES):
        sl = slice(c * S_PER_CORE, (c + 1) * S_PER_CORE)
        in_maps.append(
            {
                "m": m[sl].reshape(SLABS, V),
                "k": k[sl].reshape(SLABS, V, V),
                "diag": diag[sl].reshape(SLABS, V),
                "tgt": tgt_f[sl].reshape(SLABS, 1),
            }
        )

    res = run_bass_kernel_spmd(
        _get_nc(), in_maps, core_ids=list(range(N_CORES)), trace=trace, **run_kwargs
    )
    loss = np.concatenate(
        [r["out"].reshape(S_PER_CORE, B) for r in res.results], axis=0
    )
    return loss, res


def kernel(m, k, target):
    loss, _ = run_sharded(m, k, target)
    return loss


# revision 12
# speedup vs baseline: 1.1461x; 1.1461x over previous
"""CovQuadraticCrossEntropyLoss Trainium2 kernel.

Reference computation, per (s, b) pair with V = 512:
    p    = softmax(m)                                  [V]
    quad = 0.5 * (sum_i K_ii p_i - p^T K p)
    ce   = logsumexp(m) - m[target]
    loss = ce + quad

Strategy (memory-bound: k is 512 MB total, 64 MB per core; the roofline is
the ~179 us it takes one core to stream 64 MB of K from HBM):
  - Fully data-parallel over the s axis: core i handles s in [4i, 4i+4),
    i.e. 64 (s, b) slabs of K [512, 512] each.
  - One fused softmax pass over m [64, 512] gives e = exp(m - max) and
    Z = sum(e) (p = e / Z is never materialized; the p factors are divided
    out at the end: p^T K p = e^T K e / Z^2).
  - K slab row i lands on partition i//4, chunk i%4, so each partition's
    slab data is one contiguous 8 KB run of HBM: 128 descriptors per 1 MB
    slab DMA instead of 512.
  - Per slab: 4 accumulating float32r matmuls x[1, 512] += eT_c^T @ K_c
    compute x = K^T e at 1 cycle/row while K streams, then one DVE
    tensor_tensor_reduce against a single-partition copy of e reads x
    straight out of PSUM and lands the dot e^T K e in t_cols[0, s].
  - diag(K) is extracted host-side (a [SLABS, V] strided copy, 0.2% of K's
    bytes) and shipped as its own input: a device-side 4-byte-granule gather
    costs 32K DMA descriptors that contend with the K stream.
  - Everything not depending on x (ce, the diag term, 0.5/Z^2) is computed
    early in [64, 1] form and PE-transposed to rows, so the tail after the
    last matmul is one dot + two [1, 64] DVE ops + a 256 B output DMA.
"""

import numpy as np

import concourse.bass as bass
import concourse.mybir as mybir
import concourse.tile as tile
from concourse.masks import make_identity

S, B, V = 32, 16, 512
N_CORES = 8
S_PER_CORE = S // N_CORES          # 4
SLABS = S_PER_CORE * B             # 64 (s, b) pairs per core
P = 128                            # partitions
CHUNKS = V // P                    # 4
F32 = mybir.dt.float32
F32R = mybir.dt.float32r


def _split_multi_wait_instructions(nc: bass.Bass) -> None:
    """Rewrite the BIR so no instruction carries more than one sem wait.

    The walrus build here rejects instructions with >1 sync-wait command
    ("Too many sync wait commands", CoreV3GenImpl setupSyncWait). Engines
    execute their streams in order, so an instruction's extra waits can be
    moved onto same-engine NOPs inserted immediately before it.
    """
    for fn in nc.m.functions:
        for bb in fn.blocks:
            new_insts = []
            for inst in bb.instructions:
                si = inst.sync_info
                waits = list(si.on_wait) if si is not None and si.on_wait else []
                if len(waits) > 1:
                    for j, w in enumerate(waits[:-1]):
                        new_insts.append(
                            mybir.InstNoOp(
                                name=f"{inst.name}-sw{j}",
                                engine=inst.engine,
                                bass_nofuse=True,
                                sync_info=mybir.SyncInfo(on_wait=[w], on_update=[]),
                            )
                        )
                    inst.sync_info = mybir.SyncInfo(
                        on_wait=[waits[-1]],
                        on_update=list(si.on_update or []),
                    )
                new_insts.append(inst)
            bb.instructions = new_insts


def build_bass(k_bufs: int = 7, x_bufs: int = 6) -> bass.Bass:
    nc = bass.Bass(name="covq_ce")
    m_d = nc.dram_tensor("m", [SLABS, V], F32, kind="ExternalInput")
    k_d = nc.dram_tensor("k", [SLABS, V, V], F32, kind="ExternalInput")
    diag_d = nc.dram_tensor("diag", [SLABS, V], F32, kind="ExternalInput")
    tgt_d = nc.dram_tensor("tgt", [SLABS, 1], F32, kind="ExternalInput")
    out_d = nc.dram_tensor("out", [1, SLABS], F32, kind="ExternalOutput")

    # K slab s as [partition p, chunk c, j] with row index i = p*4 + c, so
    # partition p's line (rows 4p..4p+3) is 8 KB of contiguous HBM.
    k_r = k_d[:, :, :].rearrange("n (p c) j -> n p c j", p=P)

    with tile.TileContext(nc) as tc:
        with (
            tc.tile_pool(name="singles", bufs=1) as singles,
            tc.tile_pool(name="kpool", bufs=k_bufs) as kpool,
            tc.tile_pool(name="psum_t", bufs=1, space="PSUM") as psum_t,
            tc.tile_pool(name="psum_x", bufs=x_bufs, space="PSUM") as psum_x,
        ):
            # --- small input DMAs (SWDGE; HWDGE ring is reserved for K) ---
            m_sb = singles.tile([SLABS, V], F32)
            nc.gpsimd.dma_start(out=m_sb, in_=m_d[:, :])

            identity = singles.tile([P, P], F32)
            make_identity(nc, identity)

            diag_sb = singles.tile([SLABS, V], F32)
            nc.gpsimd.dma_start(out=diag_sb, in_=diag_d[:, :])
            iota_sb = singles.tile([SLABS, V], F32)
            nc.gpsimd.iota(
                iota_sb,
                pattern=[[1, V]],
                base=0,
                channel_multiplier=0,
                allow_small_or_imprecise_dtypes=True,
            )
            tgt_sb = singles.tile([SLABS, 1], F32)
            nc.gpsimd.dma_start(out=tgt_sb, in_=tgt_d[:, :])

            # --- softmax pieces: e = exp(m - max), Z = sum(e) --------------
            mx = singles.tile([SLABS, 1], F32)
            nc.vector.tensor_reduce(
                out=mx, in_=m_sb, axis=mybir.AxisListType.X, op=mybir.AluOpType.max
            )
            neg_mx = singles.tile([SLABS, 1], F32)
            nc.vector.tensor_scalar_mul(out=neg_mx, in0=mx, scalar1=-1.0)
            e_sb = singles.tile([SLABS, V], F32)
            z_sb = singles.tile([SLABS, 1], F32)
            nc.scalar.activation(
                out=e_sb,
                in_=m_sb,
                func=mybir.ActivationFunctionType.Exp,
                bias=neg_mx,
                scale=1.0,
                accum_out=z_sb,
            )
            ln_z = singles.tile([SLABS, 1], F32)
            nc.scalar.activation(out=ln_z, in_=z_sb, func=mybir.ActivationFunctionType.Ln)
            inv_z = singles.tile([SLABS, 1], F32)
            nc.vector.reciprocal(out=inv_z, in_=z_sb)

            # --- transpose e -> eT[p, c, s], eT_c[p, s] = e[s, 4p+c] -------
            # (matches the i = 4p+c row layout of the K tiles). float32r so
            # the matmuls take the 1-cycle/row fp32 path.
            eT_sb = singles.tile([P, CHUNKS, SLABS], F32R)
            eT_ps = psum_t.tile([P, CHUNKS, SLABS], F32)
            for c in range(CHUNKS):
                nc.tensor.transpose(
                    eT_ps[:, c, :],
                    e_sb[:, c :: CHUNKS],
                    identity[:SLABS, :SLABS],
                )
            nc.vector.tensor_copy(eT_sb, eT_ps)

            # Single-partition copy of e: row s at [0, s*V:(s+1)*V]. Lets the
            # per-slab dot product read x straight from PSUM partition 0.
            e_flat = singles.tile([1, SLABS * V], F32)
            nc.gpsimd.dma_start(out=e_flat, in_=e_sb)

            # --- early epilogue: everything that doesn't need x ------------
            # loss = (mx + lnZ - m[tgt]) + 0.5*invZ*dq - (0.5*invZ^2) * e'Ke
            #      = a - b * t      with t = e^T K e accumulated per slab.
            scratch = singles.tile([SLABS, V], F32)
            msk = singles.tile([SLABS, V], F32)
            nc.vector.tensor_scalar(
                out=msk,
                in0=iota_sb,
                scalar1=tgt_sb,
                scalar2=None,
                op0=mybir.AluOpType.is_equal,
            )
            g = singles.tile([SLABS, 1], F32)
            nc.vector.tensor_mul(out=scratch, in0=msk, in1=m_sb)
            nc.vector.tensor_reduce(out=g, in_=scratch, axis=mybir.AxisListType.X, op=mybir.AluOpType.add)
            dq = singles.tile([SLABS, 1], F32)
            nc.vector.tensor_mul(out=scratch, in0=diag_sb, in1=e_sb)
            nc.vector.tensor_reduce(out=dq, in_=scratch, axis=mybir.AxisListType.X, op=mybir.AluOpType.add)

            ce1 = singles.tile([SLABS, 1], F32)
            nc.vector.tensor_add(out=ce1, in0=mx, in1=ln_z)
            ce2 = singles.tile([SLABS, 1], F32)
            nc.vector.tensor_sub(out=ce2, in0=ce1, in1=g)
            hdq = singles.tile([SLABS, 1], F32)
            nc.vector.tensor_mul(out=hdq, in0=dq, in1=inv_z)
            hdq2 = singles.tile([SLABS, 1], F32)
            nc.vector.tensor_scalar_mul(out=hdq2, in0=hdq, scalar1=0.5)
            a_col = singles.tile([SLABS, 1], F32)
            nc.vector.tensor_add(out=a_col, in0=ce2, in1=hdq2)
            iz2 = singles.tile([SLABS, 1], F32)
            nc.vector.tensor_mul(out=iz2, in0=inv_z, in1=inv_z)
            b_col = singles.tile([SLABS, 1], F32)
            nc.vector.tensor_scalar_mul(out=b_col, in0=iz2, scalar1=0.5)

            # Rows on partition 0 (PE transpose) for the [1, 64] tail math.
            ab_row_ps = psum_t.tile([1, 2 * SLABS], F32, tag="abrow")
            a_row_ps = ab_row_ps[:, :SLABS]
            b_row_ps = ab_row_ps[:, SLABS:]
            nc.tensor.transpose(a_row_ps, a_col, identity[:SLABS, :SLABS])
            nc.tensor.transpose(b_row_ps, b_col, identity[:SLABS, :SLABS])

            # --- main loop: stream K, t[s] = e^T K e -----------------------
            t_cols = singles.tile([1, SLABS], F32)
            ttr_out = singles.tile([1, V], F32)
            for s in range(SLABS):
                kt = kpool.tile([P, CHUNKS, V], F32R, tag="kt")
                nc.sync.dma_start(out=kt, in_=k_r[s].bitcast(F32R))
                x_ps = psum_x.tile([1, V], F32, tag="x")
                for c in range(CHUNKS):
                    nc.tensor.matmul(
                        x_ps,
                        eT_sb[:, c, s : s + 1],
                        kt[:, c, :],
                        start=(c == 0),
                        stop=(c == CHUNKS - 1),
                    )
                nc.vector.tensor_mul(
                    out=ttr_out, in0=x_ps, in1=e_flat[:, s * V : (s + 1) * V]
                )
                nc.vector.tensor_reduce(
                    out=t_cols[:, s : s + 1],
                    in_=ttr_out,
                    axis=mybir.AxisListType.X,
                    op=mybir.AluOpType.add,
                )

            # --- tail: loss_row = a_row - b_row * t ------------------------
            bt = singles.tile([1, SLABS], F32)
            nc.vector.tensor_mul(out=bt, in0=b_row_ps, in1=t_cols)
            loss_row = singles.tile([1, SLABS], F32)
            nc.vector.tensor_sub(out=loss_row, in0=a_row_ps, in1=bt)

            nc.sync.dma_start(out=out_d[:, :], in_=loss_row)

    _split_multi_wait_instructions(nc)
    return nc


_NC_CACHE = {}


def _get_nc():
    if "nc" not in _NC_CACHE:
        _NC_CACHE["nc"] = build_bass()
    return _NC_CACHE["nc"]


def run_sharded(m, k, target, trace=False, **run_kwargs):
    """Shard full inputs over 8 cores, run the bass kernel, gather output.

    Returns (loss [S, B] f32, BassKernelResults).
    """
    from concourse.bass_utils import run_bass_kernel_spmd

    m = np.ascontiguousarray(np.asarray(m), dtype=np.float32)
    k = np.ascontiguousarray(np.asarray(k), dtype=np.float32)
    target = np.asarray(target)
    assert m.shape == (S, B, V) and k.shape == (S, B, V, V)
    tgt_f = target.astype(np.float32).reshape(S, B)
    diag = np.ascontiguousarray(
        np.diagonal(k, axis1=-2, axis2=-1), dtype=np.float32
    )

    in_maps = []
    for c in range(N_CORES):
        sl = slice(c * S_PER_CORE, (c + 1) * S_PER_CORE)
        in_maps.append(
            {
                "m": m[sl].reshape(SLABS, V),
                "k": k[sl].reshape(SLABS, V, V),
                "diag": diag[sl].reshape(SLABS, V),
                "tgt": tgt_f[sl].reshape(SLABS, 1),
            }
        )

    res = run_bass_kernel_spmd(
        _get_nc(), in_maps, core_ids=list(range(N_CORES)), trace=trace, **run_kwargs
    )
    loss = np.concatenate(
        [r["out"].reshape(S_PER_CORE, B) for r in res.results], axis=0
    )
    return loss, res


def kernel(m, k, target):
    loss, _ = run_sharded(m, k, target)
    return loss
